# revision 27
# baseline (speedup 1.0000x reference)
"""Mean neighbor-aggregator (segment_reduce) for TRN2, 8 NeuronCores.

out[n, :] = mean_k weight[neighbor_idx[n, k], :]      n in [0, 100000), K=10

Data-parallel over nodes: each core owns 12500 nodes (padded to 12544 =
128*98 = 98 node-tiles) plus a replicated bf16 copy of the table.

Device algorithm per core (dma_gather is limited to int16 indices, so the
100000-row table is split into 4 chunks of 25000 rows):
  - The host groups each node-tile's 1280 (node, k) slots by table chunk
    and pads every (tile, chunk) bucket to a fixed budget B[q] (pad slots
    index row 0 and carry node-id 255).  dma_gather writes position i to
    SBUF partition i%128, slot i//128, so each 128-position group is a
    [128pos, 128d] bf16 tile.
  - One dma_gather per (superbatch of 7 tiles, chunk): compact local
    indices, ~1x traffic.
  - DVE builds one-hot selection matrices A[pos, node] = (nid[pos]==node)
    by comparing a host-supplied per-position node-id lane against an
    iota constant.
  - PE accumulates out_tile[node, d] = sum_g A_g.T @ G_g in PSUM (f32),
    ACT scales by 1/K on the way out, DMA to DRAM.
"""

import numpy as np
import ml_dtypes

import concourse.bacc as bacc
import concourse.bass as bass
import concourse.mybir as mybir
import concourse.tile as tile
from concourse.bass_utils import run_bass_kernel_spmd

N_NODES = 100000
K = 10
VOCAB = 100000
D = 128
NCORES = 8
PER_CORE = N_NODES // NCORES  # 12500
P = 128
NT = 98  # node-tiles per core (12544 nodes padded)
NCHK = 4
CHK = VOCAB // NCHK  # 25000
SBT = 7  # node-tiles per superbatch
NSB = NT // SBT  # 14

BF16 = ml_dtypes.bfloat16

_CACHE = {}


def _split_multi_waits(nc):
    """walrus codegen accepts a single sync wait per instruction; hoist
    extra waits onto standalone EventSemaphore insts on the same engine."""
    for f in nc.m.functions:
        for bb in f.blocks:
            new = []
            for inst in bb.instructions:
                si = inst.sync_info
                if si is not None and si.on_wait and len(si.on_wait) > 1:
                    waits = list(si.on_wait)
                    for w in waits[:-1]:
                        nop = mybir.InstEventSemaphore(
                            name=f"wsplit-{nc.next_id()}",
                            engine=inst.engine,
                            sync_info=mybir.SyncInfo(on_wait=[w], on_update=[]),
                            ins=[],
                            outs=[],
                        )
                        nc.register_instruction(nop)
                        new.append(nop)
                    inst.sync_info = mybir.SyncInfo(
                        on_wait=[waits[-1]], on_update=list(si.on_update or [])
                    )
                new.append(inst)
            bb.instructions = new


def build(budgets):
    """budgets: tuple of 4 ints (multiple of 128), slots per (tile, chunk)."""
    f32, bf16, i16 = mybir.dt.float32, mybir.dt.bfloat16, mybir.dt.int16
    B = list(budgets)
    G = [b // P for b in B]  # groups per (tile, chunk)
    GT = sum(G)  # groups per tile
    PSB = SBT * P * GT  # positions per superbatch
    # group offset of chunk q's region within a superbatch's position space
    qgoff = np.concatenate([[0], np.cumsum([SBT * g for g in G])]).astype(int)

    nc = bacc.Bacc("TRN2", num_swdge_queues=4, dynamic_dma_scratch_size=65536)
    w_ext = nc.declare_dram_parameter("weight", [VOCAB, D], bf16, isOutput=False)
    gidx_ext = nc.declare_dram_parameter(
        "gidx", [NSB, P, PSB // 16], i16, isOutput=False
    )
    nid_ext = nc.declare_dram_parameter(
        "nid", [NSB, P, PSB // P], bf16, isOutput=False
    )
    iota_ext = nc.declare_dram_parameter("iota", [P, P], bf16, isOutput=False)
    out_ext = nc.declare_dram_parameter("out", [NT, P, D], f32, isOutput=True)

    with tile.TileContext(nc) as tc:
        with (
            tc.tile_pool(name="cst", bufs=1) as c_pool,
            tc.tile_pool(name="gb", bufs=3) as g_pool,
            tc.tile_pool(name="ab", bufs=3) as a_pool,
            tc.tile_pool(name="ob", bufs=8) as o_pool,
            tc.tile_pool(name="ps", bufs=8, space="PSUM") as p_pool,
        ):
            iota_t = c_pool.tile([P, P], bf16, name="iota")
            nc.sync.dma_start(out=iota_t[:], in_=iota_ext[:, :])
            # all superbatches' indices and node-ids in one prologue load:
            # removes the per-superbatch load from the gather critical path
            idx_all = c_pool.tile([P, NSB * (PSB // 16)], i16, name="idxall")
            # one load per superbatch into disjoint column ranges so the
            # first gather only waits ~1us for its own slice
            for sb in range(NSB):
                nc.sync.dma_start(
                    out=idx_all[:, sb * (PSB // 16) : (sb + 1) * (PSB // 16)],
                    in_=gidx_ext[sb],
                )
            nid_all = c_pool.tile([P, NSB * (PSB // P)], bf16, name="nidall")
            nc.sync.dma_start(
                out=nid_all[:],
                in_=nid_ext[:, :, :].transpose([1, 0, 2]),
            )

            for sb in range(NSB):
                ixo = sb * (PSB // 16)
                ndo = sb * (PSB // P)
                g_t = g_pool.tile([P, SBT * GT, D], bf16, tag="g", name=f"g{sb}")
                for q in range(NCHK):
                    ni = SBT * B[q]
                    nc.gpsimd.dma_gather(
                        g_t[:, qgoff[q] : qgoff[q + 1], :],
                        w_ext[q * CHK : (q + 1) * CHK, :],
                        idx_all[:, ixo + qgoff[q] * 8 : ixo + qgoff[q + 1] * 8],
                        ni,
                        ni,
                        D,
                        single_packet=False,
                        queue_num=q,
                    )

                # A[pos, node] = (nid[pos] == node), one op per chunk region
                a_t = a_pool.tile([P, SBT * GT, P], bf16, tag="a", name=f"a{sb}")
                for q in range(NCHK):
                    ng = qgoff[q + 1] - qgoff[q]
                    nc.vector.tensor_tensor(
                        out=a_t[:, qgoff[q] : qgoff[q + 1], :],
                        in0=nid_all[:, ndo + qgoff[q] : ndo + qgoff[q + 1]]
                        .unsqueeze(2)
                        .to_broadcast([P, ng, P]),
                        in1=iota_t[:].unsqueeze(1).to_broadcast([P, ng, P]),
                        op=mybir.AluOpType.is_equal,
                    )

                for ti in range(SBT):
                    t = sb * SBT + ti
                    ps_t = p_pool.tile([P, D], f32, tag="ps", name=f"ps{t}")
                    n_mm = 0
                    for q in range(NCHK):
                        for g in range(G[q]):
                            gi = qgoff[q] + ti * G[q] + g
                            nc.tensor.matmul(
                                ps_t[:],
                                lhsT=a_t[:, gi, :],
                                rhs=g_t[:, gi, :],
                                start=(n_mm == 0),
                                stop=(n_mm == GT - 1),
                            )
                            n_mm += 1
                    o_t = o_pool.tile([P, D], f32, tag="o", name=f"o{t}")
                    nc.scalar.activation(
                        out=o_t[:],
                        in_=ps_t[:],
                        func=mybir.ActivationFunctionType.Copy,
                        scale=1.0 / K,
                    )
                    nc.sync.dma_start(out=out_ext[t], in_=o_t[:])

    nc.compile()
    _split_multi_waits(nc)
    return nc


def _budgets(neighbor_idx):
    """max bucket size over (core, tile, chunk), per chunk, rounded to 128."""
    nbr = np.asarray(neighbor_idx).astype(np.int64)
    v = nbr.reshape(NCORES, PER_CORE * K)
    node = np.arange(PER_CORE).repeat(K)
    t = node // P
    q = v // CHK  # [NCORES, 125000]
    maxc = np.zeros(NCHK, dtype=np.int64)
    for c in range(NCORES):
        key = t * NCHK + q[c]
        counts = np.bincount(key, minlength=NT * NCHK).reshape(NT, NCHK)
        maxc = np.maximum(maxc, counts.max(axis=0))
    return tuple(int(-(-m // P) * P) for m in maxc)


def shard_inputs(weight, neighbor_idx, budgets):
    w_bf16 = np.ascontiguousarray(np.asarray(weight, dtype=np.float32).astype(BF16))
    nbr = np.asarray(neighbor_idx).astype(np.int64)
    B = list(budgets)
    G = [b // P for b in B]
    GT = sum(G)
    PSB = SBT * P * GT
    iota = np.ascontiguousarray(
        np.broadcast_to(np.arange(P, dtype=np.float32), (P, P)).astype(BF16)
    )

    node = np.arange(PER_CORE).repeat(K)
    t_of = node // P
    m_of = (node % P).astype(np.int32)

    in_maps = []
    for core in range(NCORES):
        v = nbr[core * PER_CORE : (core + 1) * PER_CORE].reshape(-1)
        q = (v // CHK).astype(np.int32)
        lv = (v - q * CHK).astype(np.int16)
        key = t_of * NCHK + q
        order = np.argsort(key, kind="stable")
        ks, lvs, ms = key[order], lv[order], m_of[order]
        counts = np.bincount(ks, minlength=NT * NCHK).reshape(NT, NCHK)
        seg_end = np.cumsum(counts.reshape(-1)).reshape(NT, NCHK)

        gidx = np.zeros((NSB, PSB), np.int16)
        gnid = np.full((NSB, PSB), 255.0, np.float32)
        for sb in range(NSB):
            for qq in range(NCHK):
                base = SBT * P * int(np.concatenate([[0], np.cumsum(G)])[qq])
                for ti in range(SBT):
                    t = sb * SBT + ti
                    e = seg_end[t, qq]
                    s = e - counts[t, qq]
                    n = e - s
                    pos = base + ti * B[qq]
                    gidx[sb, pos : pos + n] = lvs[s:e]
                    gnid[sb, pos : pos + n] = ms[s:e]
        # wrap idx: position i -> [i%16, i//16], replicated across 8 groups
        gidx_w = np.tile(
            gidx.reshape(NSB, PSB // 16, 16).transpose(0, 2, 1), (1, 8, 1)
        )
        # nid lanes: position i -> [i%128, i//128]
        nid_l = gnid.reshape(NSB, PSB // P, P).transpose(0, 2, 1).astype(BF16)
        in_maps.append(
            {
                "weight": w_bf16,
                "gidx": np.ascontiguousarray(gidx_w),
                "nid": np.ascontiguousarray(nid_l),
                "iota": iota,
            }
        )
    return in_maps


def unshard_output(results):
    outs = []
    for core in range(NCORES):
        o = np.asarray(results[core]["out"]).reshape(NT * P, D)[:PER_CORE]
        outs.append(o)
    return np.concatenate(outs, axis=0)


def kernel(weight, neighbor_idx):
    budgets = _budgets(neighbor_idx)
    nc = _CACHE.get(budgets)
    if nc is None:
        nc = _CACHE[budgets] = build(budgets)
    in_maps = shard_inputs(weight, neighbor_idx, budgets)
    res = run_bass_kernel_spmd(nc, in_maps, core_ids=list(range(NCORES)))
    return unshard_output(res.results)
